# revision 37
# baseline (speedup 1.0000x reference)
"""2-layer GATv2 (PyG GATv2Conv semantics) on 8 Trainium2 NeuronCores.

v2 redesign (bf16 end-to-end, tol 2e-2):
  - Nodes sharded across 8 cores (snake by in-degree); within a core, nodes
    are packed into 128-row tiles greedily balancing (lo, hi) in-edge counts
    (lo = src owned by cores 0-3) to minimize K_lo+K_hi.
  - x is pre-transposed on the host (xkT [DIN, NPC]) so layer-1 projections
    need no on-chip transposes: lhsT comes straight from SBUF.
  - xl / xl2 tables are AllGathered (2 row-chunks each, Shared outputs,
    strided out APs keeping core-major row order) and fetched per
    destination-node-tile with gpsimd dma_gather (int16 indices, lo/hi table
    halves).
  - Per node tile (T edge tiles of 128): both one-hot matrices are built with
    cheap 4x-mode tensor_scalar is_equal ops:
      oh_en[e, d] = (iota[d] == drel[e])       (edge-partition one-hot)
      oh_ne[d, e] = (dstF[d, e] == p)          (dst-partition one-hot, via a
                                                host-precomputed free-axis dst
                                                table streamed from HBM)
    z = oh_ne^T @ xr + I @ xl accumulates in PSUM (two z-batches to keep
    <= 8 banks and pipeline); scores, exp and messages run as node-tile
    batched DVE/ACT ops (att-mult at 2x, pairwise add-tree instead of the
    1x tensor_reduce); segment softmax numerator/denominator accumulate via
    oh_en^T @ [msg | ex] TensorE matmuls.
  - Softmax skips max-subtraction (scores are O(1)).
  - log_softmax over 16 output channels per node on ACT/DVE.

Full (unsharded) inputs in; full outputs out.
"""

import sys

if "/opt/trn_rl_repo" not in sys.path:
    sys.path.insert(0, "/opt/trn_rl_repo")

import numpy as np
import ml_dtypes

NC = 8          # cores
P = 128         # partitions
NEG_SLOPE = 0.2
CO2P = 128      # padded layer-2 gather row (256B in bf16)

_plan_cache = {}
_last_exec_ns = [None]
_last_debug = None


# --------------------------------------------------------------------------
# host-side graph preprocessing
# --------------------------------------------------------------------------

def _snake(order, nbins):
    n = len(order)
    ids = np.arange(n)
    round_ = ids // nbins
    pos = ids % nbins
    b = np.where(round_ % 2 == 0, pos, nbins - 1 - pos)
    out = np.empty(n, np.int64)
    out[:] = b
    return out


def _preprocess(N, E, edge_index):
    NPC = ((N + NC - 1) // NC + P - 1) // P * P    # padded nodes per core
    NT = NPC // P
    TBL = NC * NPC
    HALF = TBL // 2
    assert HALF < 32768

    src = np.concatenate([edge_index[0].astype(np.int64), np.arange(N)])
    dst = np.concatenate([edge_index[1].astype(np.int64), np.arange(N)])
    deg = np.bincount(dst, minlength=N)

    # --- core assignment: snake over degree-sorted nodes
    order = np.argsort(-deg, kind="stable")
    core_of = np.empty(N, np.int64)
    core_of[order] = _snake(order, NC)

    lo_src = core_of[src] < NC // 2                # table half of each edge
    deg_lo = np.bincount(dst[lo_src], minlength=N)
    deg_hi = deg - deg_lo

    # --- per-core tile packing: greedy, balancing (lo, hi) per tile.
    # TGT is the largest per-tile count that still fits ceil(mean/P) gather
    # tiles; crossing it is penalized so K_lo/K_hi stay at the floor.
    local_of = np.empty(N, np.int64)
    mean_half = (E + N) / 2.0 / (NC * NT)
    TGT = int(np.ceil(mean_half / P)) * P
    BIG = 1 << 40
    PEN = 1 << 20
    for c in range(NC):
        nodes = np.where(core_of == c)[0]
        nodes = nodes[np.argsort(-(deg[nodes]), kind="stable")]
        lo_cnt = np.zeros(NT, np.int64)
        hi_cnt = np.zeros(NT, np.int64)
        n_in = np.zeros(NT, np.int64)
        tile_of = np.empty(len(nodes), np.int64)
        for i, v in enumerate(nodes):
            lo2 = lo_cnt + deg_lo[v]
            hi2 = hi_cnt + deg_hi[v]
            cost = (np.maximum(lo2, hi2)
                    + PEN * (np.maximum(lo2 - TGT, 0) + np.maximum(hi2 - TGT, 0))
                    + BIG * (n_in >= P))
            tl = int(np.argmin(cost))
            tile_of[i] = tl
            lo_cnt[tl] = lo2[tl]
            hi_cnt[tl] = hi2[tl]
            n_in[tl] += 1
        # swap refinement: pads on not-full tiles count as lo edges
        pad_lo = P - n_in
        dl = deg_lo[nodes]
        dh = deg_hi[nodes]
        for _ in range(400):
            eff_lo = lo_cnt + pad_lo
            viol = np.maximum(eff_lo - TGT, 0) + np.maximum(hi_cnt - TGT, 0)
            if viol.max() == 0:
                break
            tl = int(np.argmax(viol))
            members = np.where(tile_of == tl)[0]
            best = None
            for i in members[np.argsort(-(dl[members] + dh[members]))[:24]]:
                nlo = lo_cnt - dl[i]
                nhi = hi_cnt - dh[i]
                cand = np.where(tile_of != tl)[0]
                # try swapping i with each candidate j (vectorized score)
                j_tl = tile_of[cand]
                lo_t = lo_cnt[tl] - dl[i] + dl[cand]
                hi_t = hi_cnt[tl] - dh[i] + dh[cand]
                lo_o = lo_cnt[j_tl] - dl[cand] + dl[i]
                hi_o = hi_cnt[j_tl] - dh[cand] + dh[i]
                v_new = (np.maximum(lo_t + pad_lo[tl] - TGT, 0)
                         + np.maximum(hi_t - TGT, 0)
                         + np.maximum(lo_o + pad_lo[j_tl] - TGT, 0)
                         + np.maximum(hi_o - TGT, 0))
                v_old = viol[tl] + viol[j_tl]
                gain = v_old - v_new
                jbest = int(np.argmax(gain))
                if gain[jbest] > 0 and (best is None or gain[jbest] > best[0]):
                    best = (gain[jbest], i, cand[jbest])
            if best is None:
                break
            _, i, j = best
            ti, tj = tile_of[i], tile_of[j]
            lo_cnt[ti] += dl[j] - dl[i]; hi_cnt[ti] += dh[j] - dh[i]
            lo_cnt[tj] += dl[i] - dl[j]; hi_cnt[tj] += dh[i] - dh[j]
            tile_of[i], tile_of[j] = tj, ti
        # final slot assignment
        slot = np.zeros(NT, np.int64)
        for i, v in enumerate(nodes):
            tl = tile_of[i]
            local_of[v] = tl * P + slot[tl]
            slot[tl] += 1

    r_of = core_of * NPC + local_of                # global table row

    # --- per (core, tile) edge lists split by half
    e_core = core_of[dst]
    e_tile = local_of[dst] // P
    e_slot = local_of[dst] % P
    n_in_tile = np.zeros((NC, NT), np.int64)
    for c in range(NC):
        cnt = np.bincount(local_of[core_of == c] // P, minlength=NT)
        n_in_tile[c] = cnt

    lists_lo = {}
    lists_hi = {}
    K_lo = K_hi = 1
    for c in range(NC):
        m_c = e_core == c
        for tl in range(NT):
            m = m_c & (e_tile == tl)
            ml = m & lo_src
            mh = m & ~lo_src
            lists_lo[(c, tl)] = (r_of[src[ml]], e_slot[ml])
            lists_hi[(c, tl)] = (r_of[src[mh]] - HALF, e_slot[mh])
            # fake self-edges for empty (padding) node slots keep denominators
            # nonzero; pad-slot outputs are discarded by the host
            npad = P - n_in_tile[c, tl]
            if npad:
                pads = np.arange(P - npad, P)
                a, b = lists_lo[(c, tl)]
                lists_lo[(c, tl)] = (np.concatenate([a, np.zeros(npad, np.int64)]),
                                     np.concatenate([b, pads]))
            K_lo = max(K_lo, (len(lists_lo[(c, tl)][0]) + P - 1) // P)
            K_hi = max(K_hi, (len(lists_hi[(c, tl)][0]) + P - 1) // P)

    T = K_lo + K_hi

    def pack_idx(flat):
        n = len(flat)
        s = (n + 15) // 16
        arr = np.zeros(s * 16, np.int16)
        arr[:n] = flat
        block = arr.reshape(s, 16).T
        return np.tile(block, (8, 1))

    gidx = np.zeros((NC, P, NT * T * 8), np.int16)
    drel = np.full((NC, P, NT * T), -1.0, np.float32)
    drel2 = np.full((NC, P, NT * T * 2), -1.0, ml_dtypes.bfloat16)
    dstF = np.full((NC, P, NT * T * P), -1.0, ml_dtypes.bfloat16)
    for c in range(NC):
        for tl in range(NT):
            dmat = np.full((T, P), -1.0, np.float32)
            for half, (K, k0, lst) in enumerate(
                    [(K_lo, 0, lists_lo[(c, tl)]),
                     (K_hi, K_lo, lists_hi[(c, tl)])]):
                rows, slots = lst
                n = len(rows)
                flat = np.zeros(K * P, np.int64)
                flat[:n] = rows
                off = (tl * T + k0) * 8
                gidx[c, :, off:off + K * 8] = pack_idx(flat)
                dr = np.full(K * P, -1.0, np.float32)
                dr[:n] = slots
                # drel: [P, K] layout (edge j = k*P + p -> col k, part p)
                drel[c, :, tl * T + k0: tl * T + k0 + K] = dr.reshape(K, P).T
                dmat[k0:k0 + K] = dr.reshape(K, P)
            dstF[c, :, tl * T * P:(tl + 1) * T * P] = \
                dmat.reshape(1, T * P).astype(ml_dtypes.bfloat16)
    drel2[:, :, 0::2] = drel.astype(ml_dtypes.bfloat16)
    drel2[:, :, 1::2] = drel.astype(ml_dtypes.bfloat16)

    node_order = np.full((NC, NPC), -1, np.int64)
    for c in range(NC):
        nodes = np.where(core_of == c)[0]
        node_order[c, local_of[nodes]] = nodes

    return dict(NPC=NPC, NT=NT, TBL=TBL, HALF=HALF, K_lo=K_lo, K_hi=K_hi, T=T,
                gidx=gidx, drel=drel, drel2=drel2, dstF=dstF,
                node_order=node_order,
                core_of=core_of, local_of=local_of,
                lists_lo=lists_lo, lists_hi=lists_hi)


# --------------------------------------------------------------------------
# bass program
# --------------------------------------------------------------------------

def _build_program(dims, post_passes=True):
    import concourse.bass as bass
    import concourse.mybir as mybir
    import concourse.tile as tile
    from concourse import library_config
    from concourse.bass import _add_dep_helper
    import bass_rust as _br

    fp32 = mybir.dt.float32
    bf = mybir.dt.bfloat16
    i16 = mybir.dt.int16
    AX = mybir.AxisListType
    OP = mybir.AluOpType
    AF = mybir.ActivationFunctionType

    DIN = dims["DIN"]; HC = dims["HC"]; H = dims["H"]; CH = dims["CH"]
    CO = dims["CO"]
    NPC = dims["NPC"]; NT = dims["NT"]; TBL = dims["TBL"]; HALF = dims["HALF"]
    K_lo = dims["K_lo"]; K_hi = dims["K_hi"]; T = dims["T"]
    KD = DIN // P
    KH = HC // P

    nc = bass.Bass(num_devices=NC)

    xkT = nc.dram_tensor("xkT", [DIN, NPC], bf, kind="ExternalInput")
    w1l = nc.dram_tensor("w1l", [DIN, HC], bf, kind="ExternalInput")
    w1r = nc.dram_tensor("w1r", [DIN, HC], bf, kind="ExternalInput")
    w2l = nc.dram_tensor("w2l", [HC, CO], bf, kind="ExternalInput")
    w2r = nc.dram_tensor("w2r", [HC, CO], bf, kind="ExternalInput")
    # consts: iota(P) | ident(P) | attB(HC) | att2B(CO) | iotaB(T*P)
    CCOLS = P + P + HC + CO + T * P
    consts = nc.dram_tensor("consts", [P, CCOLS], bf, kind="ExternalInput")
    constf = nc.dram_tensor("constf", [P, 2], fp32, kind="ExternalInput")
    gidx_d = nc.dram_tensor("gidx", [P, NT * T * 8], i16, kind="ExternalInput")
    drel_d = nc.dram_tensor("drel", [P, NT * T], fp32, kind="ExternalInput")
    drel2_d = nc.dram_tensor("drel2", [P, NT * T * 2], bf, kind="ExternalInput")
    dstF_d = nc.dram_tensor("dstF", [P, NT * T * P], bf, kind="ExternalInput")
    h2_out = nc.dram_tensor("h2o", [NPC, CO], fp32, kind="ExternalOutput")
    ls_out = nc.dram_tensor("lso", [NPC, CO], fp32, kind="ExternalOutput")
    DEBUG = bool(dims.get("debug"))
    if DEBUG:
        dxl = nc.dram_tensor("dxl", [NPC, HC], bf, kind="ExternalOutput")
        dxr = nc.dram_tensor("dxr", [NPC, HC], bf, kind="ExternalOutput")
        delu = nc.dram_tensor("delu", [NPC, HC], bf, kind="ExternalOutput")
        dtbl = nc.dram_tensor("dtbl", [TBL, HC], bf, kind="ExternalOutput")
        dg = nc.dram_tensor("dg", [P, T * HC], bf, kind="ExternalOutput")
        dohA = nc.dram_tensor("dohA", [P, T * P], bf, kind="ExternalOutput")
        dohB = nc.dram_tensor("dohB", [P, T * P], bf, kind="ExternalOutput")
        dtall = nc.dram_tensor("dtall", [P, T * HC], bf, kind="ExternalOutput")
        dex = nc.dram_tensor("dex", [P, T * H], bf, kind="ExternalOutput")
        dmsg = nc.dram_tensor("dmsg", [P, T * HC], bf, kind="ExternalOutput")
        dacc = nc.dram_tensor("dacc", [P, HC + H], fp32, kind="ExternalOutput")

    B1 = 4                               # z-batch slices (= 2 PSUM banks)

    with tile.TileContext(nc) as tc:
        with (
            tc.tile_pool(name="dram", bufs=1, space="DRAM") as dram,
            tc.tile_pool(name="cst", bufs=1) as cst,
        ):
            lib = nc.gpsimd.load_library(library_config.mlp)
            reg_klo = nc.gpsimd.to_reg(K_lo * P)
            reg_khi = nc.gpsimd.to_reg(K_hi * P)

            ctile = cst.tile([P, CCOLS], bf)
            nc.sync.dma_start(out=ctile[:], in_=consts[:])
            cftile = cst.tile([P, 2], fp32)
            nc.sync.dma_start(out=cftile[:], in_=constf[:])
            iota = ctile[:, 0:P]
            ident = ctile[:, P:2 * P]
            attB = ctile[:, 2 * P:2 * P + HC]
            att2B = ctile[:, 2 * P + HC:2 * P + HC + CO]
            iotaB = ctile[:, 2 * P + HC + CO:2 * P + HC + CO + T * P]
            alpha = cftile[:, 0:1]
            iotaCol = cftile[:, 1:2]

            w1l_sb = cst.tile([P, KD, HC], bf)
            w1r_sb = cst.tile([P, KD, HC], bf)
            nc.sync.dma_start(out=w1l_sb[:], in_=w1l.rearrange("(k p) c -> p k c", p=P))
            nc.sync.dma_start(out=w1r_sb[:], in_=w1r.rearrange("(k p) c -> p k c", p=P))
            w2l_sb = cst.tile([P, KH, CO], bf)
            w2r_sb = cst.tile([P, KH, CO], bf)
            nc.sync.dma_start(out=w2l_sb[:], in_=w2l.rearrange("(k p) c -> p k c", p=P))
            nc.sync.dma_start(out=w2r_sb[:], in_=w2r.rearrange("(k p) c -> p k c", p=P))

            gidx_sb = cst.tile([P, NT * T * 8], i16)
            nc.sync.dma_start(out=gidx_sb[:], in_=gidx_d[:])
            drel_sb = cst.tile([P, NT * T], fp32)
            nc.sync.dma_start(out=drel_sb[:], in_=drel_d[:])
            drel2_sb = cst.tile([P, NT * T * 2], bf)
            nc.sync.dma_start(out=drel2_sb[:], in_=drel2_d[:])

            xr1_all = cst.tile([P, NT, HC], bf)
            xr2_all = cst.tile([P, NT, CO], bf)
            h2_all = cst.tile([P, NT, CO], fp32)
            ls_all = cst.tile([P, NT, CO], fp32)

            ag1_in = dram.tile([NPC, HC], bf)
            tbl1 = dram.tile([TBL, HC], bf, addr_space="Shared")
            ag2_in = dram.tile([NPC, CO2P], bf)
            tbl2 = dram.tile([TBL, CO2P], bf, addr_space="Shared")

            xkT_r = xkT.rearrange("(k p) n -> p k n", p=P)

            # ============ phase A: layer-1 projections ============
            with (tc.tile_pool(name="sbA", bufs=3) as sb,
                  tc.tile_pool(name="psA", bufs=2, space="PSUM") as ps):
                for nt in range(NT):
                    xt = sb.tile([P, KD, P], bf, tag="xt")
                    nc.sync.dma_start(out=xt[:], in_=xkT_r[:, :, nt * P:(nt + 1) * P])
                    xl_ps = ps.tile([P, HC], fp32, tag="xl", space="PSUM")
                    xr_ps = ps.tile([P, HC], fp32, tag="xr", space="PSUM")
                    for k in range(KD):
                        nc.tensor.matmul(out=xl_ps[:], lhsT=xt[:, k, :],
                                         rhs=w1l_sb[:, k, :],
                                         start=(k == 0), stop=(k == KD - 1))
                        nc.tensor.matmul(out=xr_ps[:], lhsT=xt[:, k, :],
                                         rhs=w1r_sb[:, k, :],
                                         start=(k == 0), stop=(k == KD - 1))
                    xl_sb = sb.tile([P, HC], bf, tag="xls")
                    nc.vector.tensor_copy(out=xl_sb[:], in_=xl_ps[:])
                    nc.scalar.copy(out=xr1_all[:, nt, :], in_=xr_ps[:])
                    nc.sync.dma_start(out=ag1_in[nt * P:(nt + 1) * P, :], in_=xl_sb[:])
                    if DEBUG:
                        nc.sync.dma_start(out=dxl[nt * P:(nt + 1) * P, :],
                                          in_=xl_sb[:])

            nc.gpsimd.collective_compute(
                "AllGather", mybir.AluOpType.bypass,
                replica_groups=[list(range(NC))],
                ins=[ag1_in[:].opt()],
                outs=[tbl1[:].opt()],
            )

            # ============ phase B: layer-1 edges ============
            with (tc.tile_pool(name="sbB", bufs=2) as sb,
                  tc.tile_pool(name="psB", bufs=1, space="PSUM") as ps):
                for nt in range(NT):
                    off = nt * T * 8
                    dstF_t = sb.tile([P, T * P], bf, tag="dstF")
                    nc.sync.dma_start(out=dstF_t[:],
                                      in_=dstF_d[:, nt * T * P:(nt + 1) * T * P])
                    g = sb.tile([P, T, HC], bf, tag="g")
                    g1 = nc.gpsimd.dma_gather(
                        g[:, 0:K_lo, :], tbl1[0:HALF, :],
                        gidx_sb[:, off:off + K_lo * 8], K_lo * P, reg_klo, HC)
                    g2 = nc.gpsimd.dma_gather(
                        g[:, K_lo:T, :], tbl1[HALF:TBL, :],
                        gidx_sb[:, off + K_lo * 8:off + T * 8], K_hi * P, reg_khi, HC)
                    _add_dep_helper(g1.ins, lib.ins, sync=False, reason="lib")
                    _add_dep_helper(g2.ins, lib.ins, sync=False, reason="lib")

                    ohB = sb.tile([P, T, P], bf, tag="ohB")
                    nc.vector.tensor_scalar(
                        out=ohB[:].rearrange("p t e -> p (t e)"), in0=dstF_t[:],
                        scalar1=iotaCol, scalar2=None, op0=OP.is_equal)
                    ohA = sb.tile([P, T, P], bf, tag="ohA")
                    nc.vector.tensor_tensor(
                        out=ohA[:].rearrange("p t (dp two) -> p t dp two", two=2),
                        in0=iotaB.rearrange("p (t dp two) -> p t dp two",
                                            t=T, two=2),
                        in1=drel2_sb[:, nt * T * 2:(nt + 1) * T * 2]
                            .rearrange("p (t two) -> p t two", two=2)
                            [:, :, None, :].to_broadcast([P, T, P // 2, 2]),
                        op=OP.is_equal)

                    t_all = sb.tile([P, T, HC], bf, tag="tall")
                    acc = ps.tile([P, HC + H], fp32, tag="acc", space="PSUM")
                    for t0 in range(0, T, B1):
                        t1 = min(t0 + B1, T)
                        zb = ps.tile([P, B1, HC], fp32, tag="zb", space="PSUM",
                                     bufs=2)
                        # start=True clears has_written for the WHOLE bank, so
                        # issue it only on the first slice of each 2KB bank
                        # (two 1KB z-slices per bank).
                        for t in range(t0, t1):
                            nc.tensor.matmul(out=zb[:, t - t0, :], lhsT=ohB[:, t, :],
                                             rhs=xr1_all[:, nt, :],
                                             start=((t - t0) % 2 == 0), stop=False)
                        for t in range(t0, t1):
                            nc.tensor.matmul(out=zb[:, t - t0, :], lhsT=ident,
                                             rhs=g[:, t, :],
                                             start=False, stop=True)
                        nc.scalar.activation(out=t_all[:, t0:t1, :],
                                             in_=zb[:, 0:t1 - t0, :],
                                             func=AF.Prelu, alpha=alpha)

                    ta = sb.tile([P, T * H, CH], bf, tag="ta")
                    nc.vector.tensor_tensor(
                        out=ta[:].rearrange("p th c -> p (th c)")
                              .rearrange("p (t h c) -> p t h c", t=T, h=H),
                        in0=t_all[:].rearrange("p t (h c) -> p t h c", h=H),
                        in1=attB.rearrange("p (h c) -> p h c", h=H)[:, None, :, :]
                              .to_broadcast([P, T, H, CH]),
                        op=OP.mult)
                    tr16 = sb.tile([P, T * H, 16], bf, tag="tr16")
                    nc.vector.tensor_tensor(out=tr16[:], in0=ta[:, :, 0:16],
                                            in1=ta[:, :, 16:32], op=OP.add)
                    tr8 = sb.tile([P, T * H, 8], bf, tag="tr8")
                    nc.vector.tensor_tensor(out=tr8[:], in0=tr16[:, :, 0:8],
                                            in1=tr16[:, :, 8:16], op=OP.add)
                    sc = sb.tile([P, T * H], fp32, tag="sc")
                    nc.vector.tensor_reduce(out=sc[:], in_=tr8[:], axis=AX.X,
                                            op=OP.add)
                    ex = sb.tile([P, T, H], bf, tag="ex")
                    nc.scalar.activation(
                        out=ex[:].rearrange("p t h -> p (t h)"), in_=sc[:],
                        func=AF.Exp)
                    ex2c = sb.tile([P, T * H, 2], bf, tag="ex2c")
                    nc.vector.tensor_copy(
                        out=ex2c[:],
                        in_=ex[:].rearrange("p t h -> p (t h)")[:, :, None]
                              .to_broadcast([P, T * H, 2]))
                    msg = sb.tile([P, T * H, CH], bf, tag="msg")
                    nc.vector.tensor_tensor(
                        out=msg[:].rearrange("p th (cp two) -> p th cp two",
                                             two=2),
                        in0=g[:].rearrange("p t (h c) -> p (t h) c", h=H)
                              .rearrange("p th (cp two) -> p th cp two", two=2),
                        in1=ex2c[:, :, None, :]
                              .to_broadcast([P, T * H, CH // 2, 2]),
                        op=OP.mult)
                    # single start=True for the acc bank (num t=0); the den
                    # region's first write relies on cleared has_written bits
                    for t in range(T):
                        nc.tensor.matmul(
                            out=acc[:, 0:HC], lhsT=ohA[:, t, :],
                            rhs=msg[:, t * H:(t + 1) * H, :],
                            start=(t == 0), stop=(t == T - 1))
                        nc.tensor.matmul(
                            out=acc[:, HC:HC + H], lhsT=ohA[:, t, :],
                            rhs=ex[:, t, :],
                            start=False, stop=(t == T - 1))

                    if DEBUG and nt == 0:
                        nc.sync.dma_start(out=dg[:], in_=g[:])
                        nc.sync.dma_start(out=dohA[:], in_=ohA[:])
                        nc.sync.dma_start(out=dohB[:], in_=ohB[:])
                        nc.sync.dma_start(out=dtall[:], in_=t_all[:])
                        nc.sync.dma_start(out=dex[:], in_=ex[:])
                        nc.sync.dma_start(out=dmsg[:], in_=msg[:])
                        acc_sb = sb.tile([P, HC + H], fp32, tag="daccs")
                        nc.vector.tensor_copy(out=acc_sb[:], in_=acc[:])
                        nc.sync.dma_start(out=dacc[:], in_=acc_sb[:])
                    rec = sb.tile([P, H], fp32, tag="rec")
                    nc.vector.reciprocal(out=rec[:], in_=acc[:, HC:HC + H])
                    h1 = sb.tile([P, HC], bf, tag="h1")
                    nc.vector.tensor_tensor(
                        out=h1[:].rearrange("p (h c) -> p h c", h=H),
                        in0=acc[:, 0:HC].rearrange("p (h c) -> p h c", h=H),
                        in1=rec[:, :, None].to_broadcast([P, H, CH]),
                        op=OP.mult)
                    if dims["add_b1"]:
                        raise NotImplementedError("b1 != 0")
                    eh = sb.tile([P, HC], bf, tag="eh")
                    nc.scalar.activation(out=eh[:], in_=h1[:], func=AF.Exp)
                    em = sb.tile([P, HC], bf, tag="em")
                    nc.vector.tensor_scalar(
                        out=em[:], in0=eh[:], scalar1=1.0, scalar2=0.0,
                        op0=OP.subtract, op1=OP.min)
                    elu = sb.tile([P, HC], bf, tag="elu")
                    nc.vector.tensor_scalar(out=elu[:], in0=h1[:], scalar1=0.0,
                                            scalar2=None, op0=OP.max)
                    nc.vector.tensor_tensor(out=elu[:], in0=elu[:], in1=em[:],
                                            op=OP.add)
                    if DEBUG:
                        nc.sync.dma_start(out=delu[nt * P:(nt + 1) * P, :],
                                          in_=elu[:])

                    trn_ps = ps.tile([P, KH, P], fp32, tag="trn", space="PSUM")
                    nc.tensor.matmul(out=trn_ps[:, 0, :], lhsT=elu[:, 0:P],
                                     rhs=ident, start=True, stop=False)
                    nc.tensor.matmul(out=trn_ps[:, 1, :], lhsT=elu[:, P:2 * P],
                                     rhs=ident, start=False, stop=True)
                    hT = sb.tile([P, KH, P], bf, tag="hT")
                    nc.scalar.copy(out=hT[:], in_=trn_ps[:])
                    proj_ps = ps.tile([P, 2 * CO], fp32, tag="proj", space="PSUM")
                    for k in range(KH):
                        nc.tensor.matmul(out=proj_ps[:, 0:CO],
                                         lhsT=hT[:, k, :], rhs=w2l_sb[:, k, :],
                                         start=(k == 0), stop=(k == KH - 1))
                        nc.tensor.matmul(out=proj_ps[:, CO:2 * CO],
                                         lhsT=hT[:, k, :], rhs=w2r_sb[:, k, :],
                                         start=False, stop=(k == KH - 1))
                    xl2 = sb.tile([P, CO2P], bf, tag="xl2")
                    nc.vector.memset(xl2[:, CO:CO2P], 0.0)
                    nc.vector.tensor_copy(out=xl2[:, 0:CO],
                                          in_=proj_ps[:, 0:CO])
                    nc.scalar.copy(out=xr2_all[:, nt, :],
                                   in_=proj_ps[:, CO:2 * CO])
                    nc.sync.dma_start(out=ag2_in[nt * P:(nt + 1) * P, :],
                                      in_=xl2[:])

            nc.gpsimd.collective_compute(
                "AllGather", mybir.AluOpType.bypass,
                replica_groups=[list(range(NC))],
                ins=[ag2_in[:].opt()],
                outs=[tbl2[:].opt()],
            )

            # ============ phase C: layer-2 edges ============
            with (tc.tile_pool(name="sbC", bufs=2) as sb,
                  tc.tile_pool(name="psC", bufs=1, space="PSUM") as ps):
                for nt in range(NT):
                    off = nt * T * 8
                    dstF_t = sb.tile([P, T * P], bf, tag="dstF")
                    nc.sync.dma_start(out=dstF_t[:],
                                      in_=dstF_d[:, nt * T * P:(nt + 1) * T * P])
                    g2 = sb.tile([P, T, CO2P], bf, tag="g2")
                    c1 = nc.gpsimd.dma_gather(
                        g2[:, 0:K_lo, :], tbl2[0:HALF, :],
                        gidx_sb[:, off:off + K_lo * 8], K_lo * P, reg_klo, CO2P)
                    c2 = nc.gpsimd.dma_gather(
                        g2[:, K_lo:T, :], tbl2[HALF:TBL, :],
                        gidx_sb[:, off + K_lo * 8:off + T * 8], K_hi * P,
                        reg_khi, CO2P)
                    _add_dep_helper(c1.ins, lib.ins, sync=False, reason="lib")
                    _add_dep_helper(c2.ins, lib.ins, sync=False, reason="lib")

                    ohB = sb.tile([P, T, P], bf, tag="ohB")
                    nc.vector.tensor_scalar(
                        out=ohB[:].rearrange("p t e -> p (t e)"), in0=dstF_t[:],
                        scalar1=iotaCol, scalar2=None, op0=OP.is_equal)
                    ohA = sb.tile([P, T, P], bf, tag="ohA")
                    nc.vector.tensor_tensor(
                        out=ohA[:].rearrange("p t (dp two) -> p t dp two", two=2),
                        in0=iotaB.rearrange("p (t dp two) -> p t dp two",
                                            t=T, two=2),
                        in1=drel2_sb[:, nt * T * 2:(nt + 1) * T * 2]
                            .rearrange("p (t two) -> p t two", two=2)
                            [:, :, None, :].to_broadcast([P, T, P // 2, 2]),
                        op=OP.is_equal)

                    # z2 fits one bank: single start=True on the first matmul
                    z2 = ps.tile([P, T, CO], fp32, tag="z2", space="PSUM")
                    for t in range(T):
                        nc.tensor.matmul(out=z2[:, t, :], lhsT=ohB[:, t, :],
                                         rhs=xr2_all[:, nt, :],
                                         start=(t == 0), stop=False)
                    for t in range(T):
                        nc.tensor.matmul(out=z2[:, t, :], lhsT=ident,
                                         rhs=g2[:, t, 0:CO],
                                         start=False, stop=True)
                    t2 = sb.tile([P, T, CO], bf, tag="t2")
                    nc.scalar.activation(out=t2[:], in_=z2[:], func=AF.Prelu,
                                         alpha=alpha)
                    t2a = sb.tile([P, T, CO], bf, tag="t2a")
                    nc.vector.tensor_tensor(
                        out=t2a[:], in0=t2[:],
                        in1=att2B[:, None, :].to_broadcast([P, T, CO]),
                        op=OP.mult)
                    sc2 = sb.tile([P, T], fp32, tag="sc2")
                    nc.vector.tensor_reduce(out=sc2[:], in_=t2a[:], axis=AX.X,
                                            op=OP.add)
                    ex2 = sb.tile([P, T], bf, tag="ex2")
                    nc.scalar.activation(out=ex2[:], in_=sc2[:], func=AF.Exp)
                    msg2 = sb.tile([P, T, CO], bf, tag="msg2")
                    nc.vector.tensor_tensor(
                        out=msg2[:], in0=g2[:, :, 0:CO],
                        in1=ex2[:, :, None].to_broadcast([P, T, CO]),
                        op=OP.mult)
                    acc2 = ps.tile([P, CO + 1], fp32, tag="acc2", space="PSUM")
                    for t in range(T):
                        nc.tensor.matmul(out=acc2[:, 0:CO], lhsT=ohA[:, t, :],
                                         rhs=msg2[:, t, :],
                                         start=(t == 0), stop=(t == T - 1))
                        nc.tensor.matmul(out=acc2[:, CO:CO + 1],
                                         lhsT=ohA[:, t, :], rhs=ex2[:, t:t + 1],
                                         start=False, stop=(t == T - 1))

                    rec2 = sb.tile([P, 1], fp32, tag="rec2")
                    nc.vector.reciprocal(out=rec2[:], in_=acc2[:, CO:CO + 1])
                    nc.vector.tensor_scalar(out=h2_all[:, nt, :],
                                            in0=acc2[:, 0:CO],
                                            scalar1=rec2[:, 0:1], scalar2=None,
                                            op0=OP.mult)
                    if dims["add_b2"]:
                        raise NotImplementedError("b2 != 0")

                # batched log_softmax over all node tiles
                nmB = sb.tile([P, NT], fp32, tag="nmB")
                nc.vector.tensor_reduce(out=nmB[:], in_=h2_all[:], axis=AX.X,
                                        op=OP.max, negate=True)
                shifted = sb.tile([P, NT, CO], fp32, tag="shift")
                nc.vector.tensor_tensor(
                    out=shifted[:], in0=h2_all[:],
                    in1=nmB[:, :, None].to_broadcast([P, NT, CO]), op=OP.add)
                escB = sb.tile([P, NT, CO], bf, tag="escB")
                nc.scalar.activation(out=escB[:], in_=shifted[:], func=AF.Exp)
                ssumB = sb.tile([P, NT], fp32, tag="ssumB")
                nc.vector.tensor_reduce(out=ssumB[:], in_=escB[:], axis=AX.X,
                                        op=OP.add)
                lnsB = sb.tile([P, NT], fp32, tag="lnsB")
                nc.scalar.activation(out=lnsB[:], in_=ssumB[:], func=AF.Ln)
                nc.vector.tensor_tensor(
                    out=ls_all[:], in0=shifted[:],
                    in1=lnsB[:, :, None].to_broadcast([P, NT, CO]),
                    op=OP.subtract)

            nc.sync.dma_start(out=h2_out.rearrange("(a p) d -> p a d", p=P),
                              in_=h2_all[:])
            nc.sync.dma_start(out=ls_out.rearrange("(a p) d -> p a d", p=P),
                              in_=ls_all[:])
            if DEBUG:
                nc.sync.dma_start(out=dxr.rearrange("(a p) d -> p a d", p=P),
                                  in_=xr1_all[:])
                nc.sync.dma_start(out=dtbl[:], in_=tbl1[:])

    if post_passes:
        _br.generate_event_semaphores(nc)
        _br.codegen_inst_isa_subclasses(nc)
    return nc


# --------------------------------------------------------------------------
# entry point
# --------------------------------------------------------------------------

def kernel(x, edge_index, W1l, W1r, att1, b1, W2l, W2r, att2, b2):
    x = np.asarray(x, np.float32)
    edge_index = np.asarray(edge_index)
    W1l = np.asarray(W1l, np.float32); W1r = np.asarray(W1r, np.float32)
    att1 = np.asarray(att1, np.float32); b1 = np.asarray(b1, np.float32)
    W2l = np.asarray(W2l, np.float32); W2r = np.asarray(W2r, np.float32)
    att2 = np.asarray(att2, np.float32); b2 = np.asarray(b2, np.float32)

    N, DIN = x.shape
    E = edge_index.shape[1]
    H, CH = att1.shape
    HC = W1l.shape[1]
    CO = W2l.shape[1]

    import os
    debug = os.environ.get("GAT_DEBUG", "0") == "1"
    key = (N, E, DIN, H, CH, HC, CO, debug,
           int(np.abs(b1).max() > 0), int(np.abs(b2).max() > 0),
           hash(edge_index.tobytes()))
    if key in _plan_cache:
        pp, nc, dims, in_maps_cached = _plan_cache[key]
    else:
        pp = _preprocess(N, E, edge_index)
        dims = dict(DIN=DIN, HC=HC, H=H, CH=CH, CO=CO,
                    NPC=pp["NPC"], NT=pp["NT"], TBL=pp["TBL"],
                    HALF=pp["HALF"],
                    K_lo=pp["K_lo"], K_hi=pp["K_hi"], T=pp["T"],
                    add_b1=bool(np.abs(b1).max() > 0),
                    add_b2=bool(np.abs(b2).max() > 0),
                    debug=debug)
        nc = _build_program(dims)
        in_maps_cached = None
        _plan_cache[key] = [pp, nc, dims, None]

    NPC = pp["NPC"]
    bfdt = ml_dtypes.bfloat16

    if in_maps_cached is None:
        T = pp["T"]
        iota = np.broadcast_to(np.arange(P, dtype=np.float32)[None, :], (P, P))
        ident = np.eye(P, dtype=np.float32)
        attB = np.broadcast_to(att1.reshape(1, HC), (P, HC))
        att2B = np.broadcast_to(att2.reshape(1, CO), (P, CO))
        iotaB = np.broadcast_to(np.tile(np.arange(P, dtype=np.float32), T)[None, :],
                                (P, T * P))
        consts = np.concatenate([iota, ident, attB, att2B, iotaB],
                                axis=1).astype(bfdt)
        alpha = np.full((P, 1), NEG_SLOPE, np.float32)
        iotaCol = np.arange(P, dtype=np.float32).reshape(P, 1)
        constf = np.concatenate([alpha, iotaCol], axis=1).astype(np.float32)

        in_maps = []
        for c in range(NC):
            xkc = np.zeros((NPC, DIN), np.float32)
            sel = pp["node_order"][c]
            real = sel >= 0
            xkc[real] = x[sel[real]]
            in_maps.append(dict(
                xkT=np.ascontiguousarray(xkc.T).astype(bfdt),
                w1l=W1l.astype(bfdt), w1r=W1r.astype(bfdt),
                w2l=W2l.astype(bfdt), w2r=W2r.astype(bfdt), consts=consts,
                constf=constf,
                gidx=np.ascontiguousarray(pp["gidx"][c]),
                drel=np.ascontiguousarray(pp["drel"][c]),
                drel2=np.ascontiguousarray(pp["drel2"][c]),
                dstF=np.ascontiguousarray(pp["dstF"][c]),
            ))
        _plan_cache[key][3] = in_maps
    else:
        in_maps = in_maps_cached

    from concourse.bass_utils import run_bass_kernel_spmd
    res = run_bass_kernel_spmd(nc, in_maps, core_ids=list(range(NC)))
    _last_exec_ns[0] = getattr(res, "exec_time_ns", None)
    if debug:
        global _last_debug
        _last_debug = dict(
            tbl1=res.results[0].get("dtbl"),
            xr1=[res.results[c].get("dxr") for c in range(NC)],
            h1=[res.results[c].get("delu") for c in range(NC)],
            xl=[res.results[c].get("dxl") for c in range(NC)],
            raw0={k: v for k, v in res.results[0].items()},
        )

    h = np.empty((N, CO), np.float32)
    ls = np.empty((N, CO), np.float32)
    r_core = pp["core_of"]
    r_loc = pp["local_of"]
    for c in range(NC):
        m = r_core == c
        h[m] = res.results[c]["h2o"][r_loc[m]]
        ls[m] = res.results[c]["lso"][r_loc[m]]
    return h, ls


# revision 49
# speedup vs baseline: 1.0827x; 1.0827x over previous
"""2-layer GATv2 (PyG GATv2Conv semantics) on 8 Trainium2 NeuronCores.

v2 redesign (bf16 end-to-end, tol 2e-2):
  - Nodes sharded across 8 cores (snake by in-degree); within a core, nodes
    are packed into 128-row tiles greedily balancing (lo, hi) in-edge counts
    (lo = src owned by cores 0-3) to minimize K_lo+K_hi.
  - x is pre-transposed on the host (xkT [DIN, NPC]) so layer-1 projections
    need no on-chip transposes: lhsT comes straight from SBUF.
  - xl / xl2 tables are AllGathered (2 row-chunks each, Shared outputs,
    strided out APs keeping core-major row order) and fetched per
    destination-node-tile with gpsimd dma_gather (int16 indices, lo/hi table
    halves).
  - Per node tile (T edge tiles of 128): both one-hot matrices are built with
    cheap 4x-mode tensor_scalar is_equal ops:
      oh_en[e, d] = (iota[d] == drel[e])       (edge-partition one-hot)
      oh_ne[d, e] = (dstF[d, e] == p)          (dst-partition one-hot, via a
                                                host-precomputed free-axis dst
                                                table streamed from HBM)
    z = oh_ne^T @ xr + I @ xl accumulates in PSUM (two z-batches to keep
    <= 8 banks and pipeline); scores, exp and messages run as node-tile
    batched DVE/ACT ops (att-mult at 2x, pairwise add-tree instead of the
    1x tensor_reduce); segment softmax numerator/denominator accumulate via
    oh_en^T @ [msg | ex] TensorE matmuls.
  - Softmax skips max-subtraction (scores are O(1)).
  - log_softmax over 16 output channels per node on ACT/DVE.

Full (unsharded) inputs in; full outputs out.
"""

import sys

if "/opt/trn_rl_repo" not in sys.path:
    sys.path.insert(0, "/opt/trn_rl_repo")

import numpy as np
import ml_dtypes

NC = 8          # cores
P = 128         # partitions
NEG_SLOPE = 0.2
CO2P = 128      # padded layer-2 gather row (256B in bf16)

_plan_cache = {}
_last_exec_ns = [None]
_last_debug = None


# --------------------------------------------------------------------------
# host-side graph preprocessing
# --------------------------------------------------------------------------

def _snake(order, nbins):
    n = len(order)
    ids = np.arange(n)
    round_ = ids // nbins
    pos = ids % nbins
    b = np.where(round_ % 2 == 0, pos, nbins - 1 - pos)
    out = np.empty(n, np.int64)
    out[:] = b
    return out


def _preprocess(N, E, edge_index):
    NPC = ((N + NC - 1) // NC + P - 1) // P * P    # padded nodes per core
    NT = NPC // P
    TBL = NC * NPC
    HALF = TBL // 2
    assert HALF < 32768

    src = np.concatenate([edge_index[0].astype(np.int64), np.arange(N)])
    dst = np.concatenate([edge_index[1].astype(np.int64), np.arange(N)])
    deg = np.bincount(dst, minlength=N)

    # --- core assignment: snake over degree-sorted nodes
    order = np.argsort(-deg, kind="stable")
    core_of = np.empty(N, np.int64)
    core_of[order] = _snake(order, NC)

    lo_src = core_of[src] < NC // 2                # table half of each edge
    deg_lo = np.bincount(dst[lo_src], minlength=N)
    deg_hi = deg - deg_lo

    # --- per-core tile packing: greedy, balancing (lo, hi) per tile.
    # TGT is the largest per-tile count that still fits ceil(mean/P) gather
    # tiles; crossing it is penalized so K_lo/K_hi stay at the floor.
    local_of = np.empty(N, np.int64)
    mean_half = (E + N) / 2.0 / (NC * NT)
    TGT = int(np.ceil(mean_half / P)) * P
    BIG = 1 << 40
    PEN = 1 << 20
    for c in range(NC):
        nodes = np.where(core_of == c)[0]
        nodes = nodes[np.argsort(-(deg[nodes]), kind="stable")]
        lo_cnt = np.zeros(NT, np.int64)
        hi_cnt = np.zeros(NT, np.int64)
        n_in = np.zeros(NT, np.int64)
        tile_of = np.empty(len(nodes), np.int64)
        for i, v in enumerate(nodes):
            lo2 = lo_cnt + deg_lo[v]
            hi2 = hi_cnt + deg_hi[v]
            cost = (np.maximum(lo2, hi2)
                    + PEN * (np.maximum(lo2 - TGT, 0) + np.maximum(hi2 - TGT, 0))
                    + BIG * (n_in >= P))
            tl = int(np.argmin(cost))
            tile_of[i] = tl
            lo_cnt[tl] = lo2[tl]
            hi_cnt[tl] = hi2[tl]
            n_in[tl] += 1
        # swap refinement: pads on not-full tiles count as lo edges
        pad_lo = P - n_in
        dl = deg_lo[nodes]
        dh = deg_hi[nodes]
        for _ in range(400):
            eff_lo = lo_cnt + pad_lo
            viol = np.maximum(eff_lo - TGT, 0) + np.maximum(hi_cnt - TGT, 0)
            if viol.max() == 0:
                break
            tl = int(np.argmax(viol))
            members = np.where(tile_of == tl)[0]
            best = None
            for i in members[np.argsort(-(dl[members] + dh[members]))[:24]]:
                nlo = lo_cnt - dl[i]
                nhi = hi_cnt - dh[i]
                cand = np.where(tile_of != tl)[0]
                # try swapping i with each candidate j (vectorized score)
                j_tl = tile_of[cand]
                lo_t = lo_cnt[tl] - dl[i] + dl[cand]
                hi_t = hi_cnt[tl] - dh[i] + dh[cand]
                lo_o = lo_cnt[j_tl] - dl[cand] + dl[i]
                hi_o = hi_cnt[j_tl] - dh[cand] + dh[i]
                v_new = (np.maximum(lo_t + pad_lo[tl] - TGT, 0)
                         + np.maximum(hi_t - TGT, 0)
                         + np.maximum(lo_o + pad_lo[j_tl] - TGT, 0)
                         + np.maximum(hi_o - TGT, 0))
                v_old = viol[tl] + viol[j_tl]
                gain = v_old - v_new
                jbest = int(np.argmax(gain))
                if gain[jbest] > 0 and (best is None or gain[jbest] > best[0]):
                    best = (gain[jbest], i, cand[jbest])
            if best is None:
                break
            _, i, j = best
            ti, tj = tile_of[i], tile_of[j]
            lo_cnt[ti] += dl[j] - dl[i]; hi_cnt[ti] += dh[j] - dh[i]
            lo_cnt[tj] += dl[i] - dl[j]; hi_cnt[tj] += dh[i] - dh[j]
            tile_of[i], tile_of[j] = tj, ti
        # final slot assignment
        slot = np.zeros(NT, np.int64)
        for i, v in enumerate(nodes):
            tl = tile_of[i]
            local_of[v] = tl * P + slot[tl]
            slot[tl] += 1

    r_of = core_of * NPC + local_of                # global table row

    # --- per (core, tile) edge lists split by half
    e_core = core_of[dst]
    e_tile = local_of[dst] // P
    e_slot = local_of[dst] % P
    n_in_tile = np.zeros((NC, NT), np.int64)
    for c in range(NC):
        cnt = np.bincount(local_of[core_of == c] // P, minlength=NT)
        n_in_tile[c] = cnt

    lists_lo = {}
    lists_hi = {}
    K_lo = K_hi = 1
    for c in range(NC):
        m_c = e_core == c
        for tl in range(NT):
            m = m_c & (e_tile == tl)
            ml = m & lo_src
            mh = m & ~lo_src
            lists_lo[(c, tl)] = (r_of[src[ml]], e_slot[ml])
            lists_hi[(c, tl)] = (r_of[src[mh]] - HALF, e_slot[mh])
            # fake self-edges for empty (padding) node slots keep denominators
            # nonzero; pad-slot outputs are discarded by the host
            npad = P - n_in_tile[c, tl]
            if npad:
                pads = np.arange(P - npad, P)
                a, b = lists_lo[(c, tl)]
                lists_lo[(c, tl)] = (np.concatenate([a, np.zeros(npad, np.int64)]),
                                     np.concatenate([b, pads]))
            K_lo = max(K_lo, (len(lists_lo[(c, tl)][0]) + P - 1) // P)
            K_hi = max(K_hi, (len(lists_hi[(c, tl)][0]) + P - 1) // P)

    T = K_lo + K_hi

    def pack_idx(flat):
        n = len(flat)
        s = (n + 15) // 16
        arr = np.zeros(s * 16, np.int16)
        arr[:n] = flat
        block = arr.reshape(s, 16).T
        return np.tile(block, (8, 1))

    # node tiles per merged gather call; >1 overflows the SWDGE descriptor
    # ring (~16KB/partition) and hangs the device -- keep at 1
    G = 1
    groups = [G] * (NT // G) + ([NT % G] if NT % G else [])

    gidx = np.zeros((NC, P, NT * T * 8), np.int16)
    drel = np.full((NC, P, NT * T), -1.0, np.float32)
    drel2 = np.full((NC, P, NT * T * 2), -1.0, ml_dtypes.bfloat16)
    dstF = np.full((NC, P, NT * T * P), -1.0, ml_dtypes.bfloat16)
    for c in range(NC):
        # gidx grouped: per group of g tiles, [lo(t0)..lo(tg-1), hi(t0)..]
        base = 0
        for grp, g in enumerate(groups):
            flat_lo = np.zeros(g * K_lo * P, np.int64)
            flat_hi = np.zeros(g * K_hi * P, np.int64)
            for i in range(g):
                rows = lists_lo[(c, base + i)][0]
                flat_lo[i * K_lo * P:i * K_lo * P + len(rows)] = rows
                rows = lists_hi[(c, base + i)][0]
                flat_hi[i * K_hi * P:i * K_hi * P + len(rows)] = rows
            goff = base * T * 8
            gidx[c, :, goff:goff + g * K_lo * 8] = pack_idx(flat_lo)
            gidx[c, :, goff + g * K_lo * 8:goff + g * T * 8] = pack_idx(flat_hi)
            base += g
        for tl in range(NT):
            dmat = np.full((T, P), -1.0, np.float32)
            for half, (K, k0, lst) in enumerate(
                    [(K_lo, 0, lists_lo[(c, tl)]),
                     (K_hi, K_lo, lists_hi[(c, tl)])]):
                rows, slots = lst
                n = len(rows)
                dr = np.full(K * P, -1.0, np.float32)
                dr[:n] = slots
                # drel: [P, K] layout (edge j = k*P + p -> col k, part p)
                drel[c, :, tl * T + k0: tl * T + k0 + K] = dr.reshape(K, P).T
                dmat[k0:k0 + K] = dr.reshape(K, P)
            dstF[c, :, tl * T * P:(tl + 1) * T * P] = \
                dmat.reshape(1, T * P).astype(ml_dtypes.bfloat16)
    drel2[:, :, 0::2] = drel.astype(ml_dtypes.bfloat16)
    drel2[:, :, 1::2] = drel.astype(ml_dtypes.bfloat16)

    node_order = np.full((NC, NPC), -1, np.int64)
    for c in range(NC):
        nodes = np.where(core_of == c)[0]
        node_order[c, local_of[nodes]] = nodes

    return dict(NPC=NPC, NT=NT, TBL=TBL, HALF=HALF, K_lo=K_lo, K_hi=K_hi, T=T,
                G=G, groups=groups, gidx=gidx, drel=drel, drel2=drel2,
                dstF=dstF,
                node_order=node_order,
                core_of=core_of, local_of=local_of,
                lists_lo=lists_lo, lists_hi=lists_hi)


# --------------------------------------------------------------------------
# bass program
# --------------------------------------------------------------------------

def _build_program(dims, post_passes=True):
    import concourse.bass as bass
    import concourse.mybir as mybir
    import concourse.tile as tile
    from concourse import library_config
    from concourse.bass import _add_dep_helper
    import bass_rust as _br

    fp32 = mybir.dt.float32
    bf = mybir.dt.bfloat16
    i16 = mybir.dt.int16
    AX = mybir.AxisListType
    OP = mybir.AluOpType
    AF = mybir.ActivationFunctionType

    DIN = dims["DIN"]; HC = dims["HC"]; H = dims["H"]; CH = dims["CH"]
    CO = dims["CO"]
    NPC = dims["NPC"]; NT = dims["NT"]; TBL = dims["TBL"]; HALF = dims["HALF"]
    K_lo = dims["K_lo"]; K_hi = dims["K_hi"]; T = dims["T"]
    KD = DIN // P
    KH = HC // P

    nc = bass.Bass(num_devices=NC)

    xkT = nc.dram_tensor("xkT", [DIN, NPC], bf, kind="ExternalInput")
    w1l = nc.dram_tensor("w1l", [DIN, HC], bf, kind="ExternalInput")
    w1r = nc.dram_tensor("w1r", [DIN, HC], bf, kind="ExternalInput")
    w2l = nc.dram_tensor("w2l", [HC, CO], bf, kind="ExternalInput")
    w2r = nc.dram_tensor("w2r", [HC, CO], bf, kind="ExternalInput")
    # consts: iota(P) | ident(P) | attB(HC) | att2B(CO) | iotaB(T*P)
    CCOLS = P + P + HC + CO + T * P
    consts = nc.dram_tensor("consts", [P, CCOLS], bf, kind="ExternalInput")
    constf = nc.dram_tensor("constf", [P, 2], fp32, kind="ExternalInput")
    gidx_d = nc.dram_tensor("gidx", [P, NT * T * 8], i16, kind="ExternalInput")
    drel_d = nc.dram_tensor("drel", [P, NT * T], fp32, kind="ExternalInput")
    drel2_d = nc.dram_tensor("drel2", [P, NT * T * 2], bf, kind="ExternalInput")
    dstF_d = nc.dram_tensor("dstF", [P, NT * T * P], bf, kind="ExternalInput")
    h2_out = nc.dram_tensor("h2o", [NPC, CO], fp32, kind="ExternalOutput")
    ls_out = nc.dram_tensor("lso", [NPC, CO], fp32, kind="ExternalOutput")
    DEBUG = bool(dims.get("debug"))
    if DEBUG:
        dxl = nc.dram_tensor("dxl", [NPC, HC], bf, kind="ExternalOutput")
        dxr = nc.dram_tensor("dxr", [NPC, HC], bf, kind="ExternalOutput")
        delu = nc.dram_tensor("delu", [NPC, HC], bf, kind="ExternalOutput")
        dtbl = nc.dram_tensor("dtbl", [TBL, HC], bf, kind="ExternalOutput")
        dg = nc.dram_tensor("dg", [P, T * HC], bf, kind="ExternalOutput")
        dohA = nc.dram_tensor("dohA", [P, T * P], bf, kind="ExternalOutput")
        dohB = nc.dram_tensor("dohB", [P, T * P], bf, kind="ExternalOutput")
        dtall = nc.dram_tensor("dtall", [P, T * HC], bf, kind="ExternalOutput")
        dex = nc.dram_tensor("dex", [P, T * H], bf, kind="ExternalOutput")
        dmsg = nc.dram_tensor("dmsg", [P, T * HC], bf, kind="ExternalOutput")
        dacc = nc.dram_tensor("dacc", [P, HC + H], fp32, kind="ExternalOutput")

    B1 = 4                               # z-batch slices (= 2 PSUM banks)

    with tile.TileContext(nc) as tc:
        with (
            tc.tile_pool(name="dram", bufs=1, space="DRAM") as dram,
            tc.tile_pool(name="cst", bufs=1) as cst,
        ):
            lib = nc.gpsimd.load_library(library_config.mlp)
            reg_klo = nc.gpsimd.to_reg(K_lo * P)
            reg_khi = nc.gpsimd.to_reg(K_hi * P)

            ctile = cst.tile([P, CCOLS], bf)
            nc.sync.dma_start(out=ctile[:], in_=consts[:])
            cftile = cst.tile([P, 2], fp32)
            nc.sync.dma_start(out=cftile[:], in_=constf[:])
            iota = ctile[:, 0:P]
            ident = ctile[:, P:2 * P]
            attB = ctile[:, 2 * P:2 * P + HC]
            att2B = ctile[:, 2 * P + HC:2 * P + HC + CO]
            iotaB = ctile[:, 2 * P + HC + CO:2 * P + HC + CO + T * P]
            alpha = cftile[:, 0:1]
            iotaCol = cftile[:, 1:2]

            w1l_sb = cst.tile([P, KD, HC], bf)
            w1r_sb = cst.tile([P, KD, HC], bf)
            nc.sync.dma_start(out=w1l_sb[:], in_=w1l.rearrange("(k p) c -> p k c", p=P))
            nc.sync.dma_start(out=w1r_sb[:], in_=w1r.rearrange("(k p) c -> p k c", p=P))
            w2l_sb = cst.tile([P, KH, CO], bf)
            w2r_sb = cst.tile([P, KH, CO], bf)
            nc.sync.dma_start(out=w2l_sb[:], in_=w2l.rearrange("(k p) c -> p k c", p=P))
            nc.sync.dma_start(out=w2r_sb[:], in_=w2r.rearrange("(k p) c -> p k c", p=P))

            gidx_sb = cst.tile([P, NT * T * 8], i16)
            nc.sync.dma_start(out=gidx_sb[:], in_=gidx_d[:])
            drel_sb = cst.tile([P, NT * T], fp32)
            nc.sync.dma_start(out=drel_sb[:], in_=drel_d[:])
            drel2_sb = cst.tile([P, NT * T * 2], bf)
            nc.sync.dma_start(out=drel2_sb[:], in_=drel2_d[:])

            xr1_all = cst.tile([P, NT, HC], bf)
            xr2_all = cst.tile([P, NT, CO], bf)
            h2_all = cst.tile([P, NT, CO], fp32)
            ls_all = cst.tile([P, NT, CO], fp32)

            ag1_in = dram.tile([NPC, HC], bf)
            tbl1 = dram.tile([TBL, HC], bf, addr_space="Shared")
            ag2_in = dram.tile([NPC, CO2P], bf)
            tbl2 = dram.tile([TBL, CO2P], bf, addr_space="Shared")

            xkT_r = xkT.rearrange("(k p) n -> p k n", p=P)

            # ============ phase A: layer-1 projections ============
            with (tc.tile_pool(name="sbA", bufs=3) as sb,
                  tc.tile_pool(name="psA", bufs=2, space="PSUM") as ps):
                for nt in range(NT):
                    xt = sb.tile([P, KD, P], bf, tag="xt")
                    nc.sync.dma_start(out=xt[:], in_=xkT_r[:, :, nt * P:(nt + 1) * P])
                    xl_ps = ps.tile([P, HC], fp32, tag="xl", space="PSUM")
                    xr_ps = ps.tile([P, HC], fp32, tag="xr", space="PSUM")
                    for k in range(KD):
                        nc.tensor.matmul(out=xl_ps[:], lhsT=xt[:, k, :],
                                         rhs=w1l_sb[:, k, :],
                                         start=(k == 0), stop=(k == KD - 1))
                        nc.tensor.matmul(out=xr_ps[:], lhsT=xt[:, k, :],
                                         rhs=w1r_sb[:, k, :],
                                         start=(k == 0), stop=(k == KD - 1))
                    xl_sb = sb.tile([P, HC], bf, tag="xls")
                    nc.vector.tensor_copy(out=xl_sb[:], in_=xl_ps[:])
                    nc.scalar.copy(out=xr1_all[:, nt, :], in_=xr_ps[:])
                    nc.sync.dma_start(out=ag1_in[nt * P:(nt + 1) * P, :], in_=xl_sb[:])
                    if DEBUG:
                        nc.sync.dma_start(out=dxl[nt * P:(nt + 1) * P, :],
                                          in_=xl_sb[:])

            nc.gpsimd.collective_compute(
                "AllGather", mybir.AluOpType.bypass,
                replica_groups=[list(range(NC))],
                ins=[ag1_in[:].opt()],
                outs=[tbl1[:].opt()],
            )

            # ============ phase B: layer-1 edges ============
            G = dims["G"]
            groups = dims["groups"]
            regs = {g: (nc.gpsimd.to_reg(g * K_lo * P),
                        nc.gpsimd.to_reg(g * K_hi * P)) for g in set(groups)}
            with (tc.tile_pool(name="sbB", bufs=2) as sb,
                  tc.tile_pool(name="psB", bufs=1, space="PSUM") as ps):
              base = 0
              for grp, g in enumerate(groups):
                goff = base * T * 8
                glo = sb.tile([P, G * K_lo, HC], bf, tag="glo")
                ghi = sb.tile([P, G * K_hi, HC], bf, tag="ghi")
                gB1 = nc.gpsimd.dma_gather(
                    glo[:, 0:g * K_lo, :], tbl1[0:HALF, :],
                    gidx_sb[:, goff:goff + g * K_lo * 8],
                    g * K_lo * P, regs[g][0], HC)
                gB2 = nc.gpsimd.dma_gather(
                    ghi[:, 0:g * K_hi, :], tbl1[HALF:TBL, :],
                    gidx_sb[:, goff + g * K_lo * 8:goff + g * T * 8],
                    g * K_hi * P, regs[g][1], HC)
                _add_dep_helper(gB1.ins, lib.ins, sync=False, reason="lib")
                _add_dep_helper(gB2.ins, lib.ins, sync=False, reason="lib")
                for i in range(g):
                    nt = base + i
                    g_lo = glo[:, i * K_lo:(i + 1) * K_lo, :]
                    g_hi = ghi[:, i * K_hi:(i + 1) * K_hi, :]
                    dstF_t = sb.tile([P, T * P], bf, tag="dstF")
                    nc.sync.dma_start(out=dstF_t[:],
                                      in_=dstF_d[:, nt * T * P:(nt + 1) * T * P])

                    ohB = sb.tile([P, T, P], bf, tag="ohB")
                    nc.vector.tensor_scalar(
                        out=ohB[:].rearrange("p t e -> p (t e)"), in0=dstF_t[:],
                        scalar1=iotaCol, scalar2=None, op0=OP.is_equal)
                    ohA = sb.tile([P, T, P], bf, tag="ohA")
                    nc.vector.tensor_tensor(
                        out=ohA[:].rearrange("p t (dp two) -> p t dp two", two=2),
                        in0=iotaB.rearrange("p (t dp two) -> p t dp two",
                                            t=T, two=2),
                        in1=drel2_sb[:, nt * T * 2:(nt + 1) * T * 2]
                            .rearrange("p (t two) -> p t two", two=2)
                            [:, :, None, :].to_broadcast([P, T, P // 2, 2]),
                        op=OP.is_equal)

                    t_all = sb.tile([P, T, HC], bf, tag="tall")
                    acc = ps.tile([P, HC + H], fp32, tag="acc", space="PSUM")
                    for t0 in range(0, T, B1):
                        t1 = min(t0 + B1, T)
                        zb = ps.tile([P, B1, HC], fp32, tag="zb", space="PSUM",
                                     bufs=2)
                        # start=True clears has_written for the WHOLE bank, so
                        # issue it only on the first slice of each 2KB bank
                        # (two 1KB z-slices per bank).
                        for t in range(t0, t1):
                            nc.tensor.matmul(out=zb[:, t - t0, :], lhsT=ohB[:, t, :],
                                             rhs=xr1_all[:, nt, :],
                                             start=((t - t0) % 2 == 0), stop=False)
                        for t in range(t0, t1):
                            xl_t = (g_lo[:, t, :] if t < K_lo
                                    else g_hi[:, t - K_lo, :])
                            nc.tensor.matmul(out=zb[:, t - t0, :], lhsT=ident,
                                             rhs=xl_t,
                                             start=False, stop=True)
                        nc.scalar.activation(out=t_all[:, t0:t1, :],
                                             in_=zb[:, 0:t1 - t0, :],
                                             func=AF.Prelu, alpha=alpha)

                    # att-scale in place over t_all (t is not needed after)
                    ta = t_all[:].rearrange("p t (th c) -> p (t th) c", th=H)
                    nc.vector.tensor_tensor(
                        out=t_all[:].rearrange("p t (h c) -> p t h c", h=H),
                        in0=t_all[:].rearrange("p t (h c) -> p t h c", h=H),
                        in1=attB.rearrange("p (h c) -> p h c", h=H)[:, None, :, :]
                              .to_broadcast([P, T, H, CH]),
                        op=OP.mult)
                    tr16 = sb.tile([P, T * H, 16], bf, tag="tr16")
                    nc.vector.tensor_tensor(out=tr16[:], in0=ta[:, :, 0:16],
                                            in1=ta[:, :, 16:32], op=OP.add)
                    tr8 = sb.tile([P, T * H, 8], bf, tag="tr8")
                    nc.vector.tensor_tensor(out=tr8[:], in0=tr16[:, :, 0:8],
                                            in1=tr16[:, :, 8:16], op=OP.add)
                    sc = sb.tile([P, T * H], fp32, tag="sc")
                    nc.vector.tensor_reduce(out=sc[:], in_=tr8[:], axis=AX.X,
                                            op=OP.add)
                    ex = sb.tile([P, T, H], bf, tag="ex")
                    nc.scalar.activation(
                        out=ex[:].rearrange("p t h -> p (t h)"), in_=sc[:],
                        func=AF.Exp)
                    ex2c = sb.tile([P, T * H, 2], bf, tag="ex2c")
                    nc.vector.tensor_copy(
                        out=ex2c[:],
                        in_=ex[:].rearrange("p t h -> p (t h)")[:, :, None]
                              .to_broadcast([P, T * H, 2]))
                    msg = sb.tile([P, T * H, CH], bf, tag="msg")
                    nc.vector.tensor_tensor(
                        out=msg[:, 0:K_lo * H, :]
                              .rearrange("p th (cp two) -> p th cp two", two=2),
                        in0=g_lo[:].rearrange("p t (h c) -> p (t h) c", h=H)
                              .rearrange("p th (cp two) -> p th cp two", two=2),
                        in1=ex2c[:, 0:K_lo * H, None, :]
                              .to_broadcast([P, K_lo * H, CH // 2, 2]),
                        op=OP.mult)
                    nc.vector.tensor_tensor(
                        out=msg[:, K_lo * H:T * H, :]
                              .rearrange("p th (cp two) -> p th cp two", two=2),
                        in0=g_hi[:].rearrange("p t (h c) -> p (t h) c", h=H)
                              .rearrange("p th (cp two) -> p th cp two", two=2),
                        in1=ex2c[:, K_lo * H:T * H, None, :]
                              .to_broadcast([P, K_hi * H, CH // 2, 2]),
                        op=OP.mult)
                    # single start=True for the acc bank (num t=0); the den
                    # region's first write relies on cleared has_written bits
                    for t in range(T):
                        nc.tensor.matmul(
                            out=acc[:, 0:HC], lhsT=ohA[:, t, :],
                            rhs=msg[:, t * H:(t + 1) * H, :],
                            start=(t == 0), stop=(t == T - 1))
                        nc.tensor.matmul(
                            out=acc[:, HC:HC + H], lhsT=ohA[:, t, :],
                            rhs=ex[:, t, :],
                            start=False, stop=(t == T - 1))

                    if DEBUG and nt == 0:
                        nc.sync.dma_start(out=dg[:], in_=glo[:, 0:K_lo, :])
                        nc.sync.dma_start(out=dohA[:], in_=ohA[:])
                        nc.sync.dma_start(out=dohB[:], in_=ohB[:])
                        nc.sync.dma_start(out=dtall[:], in_=t_all[:])
                        nc.sync.dma_start(out=dex[:], in_=ex[:])
                        nc.sync.dma_start(out=dmsg[:], in_=msg[:])
                        acc_sb = sb.tile([P, HC + H], fp32, tag="daccs")
                        nc.vector.tensor_copy(out=acc_sb[:], in_=acc[:])
                        nc.sync.dma_start(out=dacc[:], in_=acc_sb[:])
                    rec = sb.tile([P, H], fp32, tag="rec")
                    nc.vector.reciprocal(out=rec[:], in_=acc[:, HC:HC + H])
                    h1 = sb.tile([P, HC], bf, tag="h1")
                    nc.vector.tensor_tensor(
                        out=h1[:].rearrange("p (h c) -> p h c", h=H),
                        in0=acc[:, 0:HC].rearrange("p (h c) -> p h c", h=H),
                        in1=rec[:, :, None].to_broadcast([P, H, CH]),
                        op=OP.mult)
                    if dims["add_b1"]:
                        raise NotImplementedError("b1 != 0")
                    eh = sb.tile([P, HC], bf, tag="eh")
                    nc.scalar.activation(out=eh[:], in_=h1[:], func=AF.Exp)
                    em = sb.tile([P, HC], bf, tag="em")
                    nc.vector.tensor_scalar(
                        out=em[:], in0=eh[:], scalar1=1.0, scalar2=0.0,
                        op0=OP.subtract, op1=OP.min)
                    elu = sb.tile([P, HC], bf, tag="elu")
                    nc.vector.tensor_scalar(out=elu[:], in0=h1[:], scalar1=0.0,
                                            scalar2=None, op0=OP.max)
                    nc.vector.tensor_tensor(out=elu[:], in0=elu[:], in1=em[:],
                                            op=OP.add)
                    if DEBUG:
                        nc.sync.dma_start(out=delu[nt * P:(nt + 1) * P, :],
                                          in_=elu[:])

                    trn_ps = ps.tile([P, KH, P], fp32, tag="trn", space="PSUM")
                    nc.tensor.matmul(out=trn_ps[:, 0, :], lhsT=elu[:, 0:P],
                                     rhs=ident, start=True, stop=False)
                    nc.tensor.matmul(out=trn_ps[:, 1, :], lhsT=elu[:, P:2 * P],
                                     rhs=ident, start=False, stop=True)
                    hT = sb.tile([P, KH, P], bf, tag="hT")
                    nc.scalar.copy(out=hT[:], in_=trn_ps[:])
                    proj_ps = ps.tile([P, 2 * CO], fp32, tag="proj", space="PSUM")
                    for k in range(KH):
                        nc.tensor.matmul(out=proj_ps[:, 0:CO],
                                         lhsT=hT[:, k, :], rhs=w2l_sb[:, k, :],
                                         start=(k == 0), stop=(k == KH - 1))
                        nc.tensor.matmul(out=proj_ps[:, CO:2 * CO],
                                         lhsT=hT[:, k, :], rhs=w2r_sb[:, k, :],
                                         start=False, stop=(k == KH - 1))
                    xl2 = sb.tile([P, CO2P], bf, tag="xl2")
                    nc.vector.memset(xl2[:, CO:CO2P], 0.0)
                    nc.vector.tensor_copy(out=xl2[:, 0:CO],
                                          in_=proj_ps[:, 0:CO])
                    nc.scalar.copy(out=xr2_all[:, nt, :],
                                   in_=proj_ps[:, CO:2 * CO])
                    nc.sync.dma_start(out=ag2_in[nt * P:(nt + 1) * P, :],
                                      in_=xl2[:])
                base += g

            nc.gpsimd.collective_compute(
                "AllGather", mybir.AluOpType.bypass,
                replica_groups=[list(range(NC))],
                ins=[ag2_in[:].opt()],
                outs=[tbl2[:].opt()],
            )

            # ============ phase C: layer-2 edges ============
            with (tc.tile_pool(name="sbC", bufs=2) as sb,
                  tc.tile_pool(name="psC", bufs=1, space="PSUM") as ps):
              base = 0
              for grp, g in enumerate(groups):
                goff = base * T * 8
                g2lo = sb.tile([P, G * K_lo, CO2P], bf, tag="g2lo")
                g2hi = sb.tile([P, G * K_hi, CO2P], bf, tag="g2hi")
                cB1 = nc.gpsimd.dma_gather(
                    g2lo[:, 0:g * K_lo, :], tbl2[0:HALF, :],
                    gidx_sb[:, goff:goff + g * K_lo * 8],
                    g * K_lo * P, regs[g][0], CO2P)
                cB2 = nc.gpsimd.dma_gather(
                    g2hi[:, 0:g * K_hi, :], tbl2[HALF:TBL, :],
                    gidx_sb[:, goff + g * K_lo * 8:goff + g * T * 8],
                    g * K_hi * P, regs[g][1], CO2P)
                _add_dep_helper(cB1.ins, lib.ins, sync=False, reason="lib")
                _add_dep_helper(cB2.ins, lib.ins, sync=False, reason="lib")
                for i in range(g):
                    nt = base + i
                    g2_lo = g2lo[:, i * K_lo:(i + 1) * K_lo, :]
                    g2_hi = g2hi[:, i * K_hi:(i + 1) * K_hi, :]
                    dstF_t = sb.tile([P, T * P], bf, tag="dstF")
                    nc.sync.dma_start(out=dstF_t[:],
                                      in_=dstF_d[:, nt * T * P:(nt + 1) * T * P])

                    ohB = sb.tile([P, T, P], bf, tag="ohB")
                    nc.vector.tensor_scalar(
                        out=ohB[:].rearrange("p t e -> p (t e)"), in0=dstF_t[:],
                        scalar1=iotaCol, scalar2=None, op0=OP.is_equal)
                    ohA = sb.tile([P, T, P], bf, tag="ohA")
                    nc.vector.tensor_tensor(
                        out=ohA[:].rearrange("p t (dp two) -> p t dp two", two=2),
                        in0=iotaB.rearrange("p (t dp two) -> p t dp two",
                                            t=T, two=2),
                        in1=drel2_sb[:, nt * T * 2:(nt + 1) * T * 2]
                            .rearrange("p (t two) -> p t two", two=2)
                            [:, :, None, :].to_broadcast([P, T, P // 2, 2]),
                        op=OP.is_equal)

                    # z2 fits one bank: single start=True on the first matmul
                    z2 = ps.tile([P, T, CO], fp32, tag="z2", space="PSUM")
                    for t in range(T):
                        nc.tensor.matmul(out=z2[:, t, :], lhsT=ohB[:, t, :],
                                         rhs=xr2_all[:, nt, :],
                                         start=(t == 0), stop=False)
                    for t in range(T):
                        xl2_t = (g2_lo[:, t, 0:CO] if t < K_lo
                                 else g2_hi[:, t - K_lo, 0:CO])
                        nc.tensor.matmul(out=z2[:, t, :], lhsT=ident,
                                         rhs=xl2_t,
                                         start=False, stop=True)
                    t2 = sb.tile([P, T, CO], bf, tag="t2")
                    nc.scalar.activation(out=t2[:], in_=z2[:], func=AF.Prelu,
                                         alpha=alpha)
                    t2a = sb.tile([P, T, CO], bf, tag="t2a")
                    nc.vector.tensor_tensor(
                        out=t2a[:], in0=t2[:],
                        in1=att2B[:, None, :].to_broadcast([P, T, CO]),
                        op=OP.mult)
                    sc2 = sb.tile([P, T], fp32, tag="sc2")
                    nc.vector.tensor_reduce(out=sc2[:], in_=t2a[:], axis=AX.X,
                                            op=OP.add)
                    ex2 = sb.tile([P, T], bf, tag="ex2")
                    nc.scalar.activation(out=ex2[:], in_=sc2[:], func=AF.Exp)
                    # msg2: [P, T, 17] = [xl2*ex2 | ex2]; one 17-col matmul
                    # per edge tile covers numerator + denominator
                    msg2 = sb.tile([P, T, CO + 1], bf, tag="msg2")
                    nc.vector.tensor_tensor(
                        out=msg2[:, 0:K_lo, 0:CO], in0=g2_lo[:, :, 0:CO],
                        in1=ex2[:, 0:K_lo, None].to_broadcast([P, K_lo, CO]),
                        op=OP.mult)
                    nc.vector.tensor_tensor(
                        out=msg2[:, K_lo:T, 0:CO], in0=g2_hi[:, :, 0:CO],
                        in1=ex2[:, K_lo:T, None].to_broadcast([P, K_hi, CO]),
                        op=OP.mult)
                    nc.vector.tensor_copy(out=msg2[:, :, CO:CO + 1],
                                          in_=ex2[:, :, None])
                    acc2 = ps.tile([P, CO + 1], fp32, tag="acc2", space="PSUM")
                    for t in range(T):
                        nc.tensor.matmul(out=acc2[:], lhsT=ohA[:, t, :],
                                         rhs=msg2[:, t, :],
                                         start=(t == 0), stop=(t == T - 1))

                    rec2 = sb.tile([P, 1], fp32, tag="rec2")
                    nc.vector.reciprocal(out=rec2[:], in_=acc2[:, CO:CO + 1])
                    nc.vector.tensor_scalar(out=h2_all[:, nt, :],
                                            in0=acc2[:, 0:CO],
                                            scalar1=rec2[:, 0:1], scalar2=None,
                                            op0=OP.mult)
                    if dims["add_b2"]:
                        raise NotImplementedError("b2 != 0")
                base += g

              with tc.tile_pool(name="sbC2", bufs=1) as sb:
                # batched log_softmax over all node tiles
                nmB = sb.tile([P, NT], fp32, tag="nmB")
                nc.vector.tensor_reduce(out=nmB[:], in_=h2_all[:], axis=AX.X,
                                        op=OP.max, negate=True)
                shifted = sb.tile([P, NT, CO], fp32, tag="shift")
                nc.vector.tensor_tensor(
                    out=shifted[:], in0=h2_all[:],
                    in1=nmB[:, :, None].to_broadcast([P, NT, CO]), op=OP.add)
                escB = sb.tile([P, NT, CO], bf, tag="escB")
                nc.scalar.activation(out=escB[:], in_=shifted[:], func=AF.Exp)
                ssumB = sb.tile([P, NT], fp32, tag="ssumB")
                nc.vector.tensor_reduce(out=ssumB[:], in_=escB[:], axis=AX.X,
                                        op=OP.add)
                lnsB = sb.tile([P, NT], fp32, tag="lnsB")
                nc.scalar.activation(out=lnsB[:], in_=ssumB[:], func=AF.Ln)
                nc.vector.tensor_tensor(
                    out=ls_all[:], in0=shifted[:],
                    in1=lnsB[:, :, None].to_broadcast([P, NT, CO]),
                    op=OP.subtract)

            nc.sync.dma_start(out=h2_out.rearrange("(a p) d -> p a d", p=P),
                              in_=h2_all[:])
            nc.sync.dma_start(out=ls_out.rearrange("(a p) d -> p a d", p=P),
                              in_=ls_all[:])
            if DEBUG:
                nc.sync.dma_start(out=dxr.rearrange("(a p) d -> p a d", p=P),
                                  in_=xr1_all[:])
                nc.sync.dma_start(out=dtbl[:], in_=tbl1[:])

    if post_passes:
        _br.generate_event_semaphores(nc)
        _br.codegen_inst_isa_subclasses(nc)
    return nc


# --------------------------------------------------------------------------
# entry point
# --------------------------------------------------------------------------

def kernel(x, edge_index, W1l, W1r, att1, b1, W2l, W2r, att2, b2):
    x = np.asarray(x, np.float32)
    edge_index = np.asarray(edge_index)
    W1l = np.asarray(W1l, np.float32); W1r = np.asarray(W1r, np.float32)
    att1 = np.asarray(att1, np.float32); b1 = np.asarray(b1, np.float32)
    W2l = np.asarray(W2l, np.float32); W2r = np.asarray(W2r, np.float32)
    att2 = np.asarray(att2, np.float32); b2 = np.asarray(b2, np.float32)

    N, DIN = x.shape
    E = edge_index.shape[1]
    H, CH = att1.shape
    HC = W1l.shape[1]
    CO = W2l.shape[1]

    import os
    debug = os.environ.get("GAT_DEBUG", "0") == "1"
    key = (N, E, DIN, H, CH, HC, CO, debug,
           int(np.abs(b1).max() > 0), int(np.abs(b2).max() > 0),
           hash(edge_index.tobytes()))
    if key in _plan_cache:
        pp, nc, dims, in_maps_cached = _plan_cache[key]
    else:
        pp = _preprocess(N, E, edge_index)
        dims = dict(DIN=DIN, HC=HC, H=H, CH=CH, CO=CO,
                    NPC=pp["NPC"], NT=pp["NT"], TBL=pp["TBL"],
                    HALF=pp["HALF"],
                    K_lo=pp["K_lo"], K_hi=pp["K_hi"], T=pp["T"], G=pp["G"],
                    groups=pp["groups"],
                    add_b1=bool(np.abs(b1).max() > 0),
                    add_b2=bool(np.abs(b2).max() > 0),
                    debug=debug)
        nc = _build_program(dims)
        in_maps_cached = None
        _plan_cache[key] = [pp, nc, dims, None]

    NPC = pp["NPC"]
    bfdt = ml_dtypes.bfloat16

    if in_maps_cached is None:
        T = pp["T"]
        iota = np.broadcast_to(np.arange(P, dtype=np.float32)[None, :], (P, P))
        ident = np.eye(P, dtype=np.float32)
        attB = np.broadcast_to(att1.reshape(1, HC), (P, HC))
        att2B = np.broadcast_to(att2.reshape(1, CO), (P, CO))
        iotaB = np.broadcast_to(np.tile(np.arange(P, dtype=np.float32), T)[None, :],
                                (P, T * P))
        consts = np.concatenate([iota, ident, attB, att2B, iotaB],
                                axis=1).astype(bfdt)
        alpha = np.full((P, 1), NEG_SLOPE, np.float32)
        iotaCol = np.arange(P, dtype=np.float32).reshape(P, 1)
        constf = np.concatenate([alpha, iotaCol], axis=1).astype(np.float32)

        in_maps = []
        for c in range(NC):
            xkc = np.zeros((NPC, DIN), np.float32)
            sel = pp["node_order"][c]
            real = sel >= 0
            xkc[real] = x[sel[real]]
            in_maps.append(dict(
                xkT=np.ascontiguousarray(xkc.T).astype(bfdt),
                w1l=W1l.astype(bfdt), w1r=W1r.astype(bfdt),
                w2l=W2l.astype(bfdt), w2r=W2r.astype(bfdt), consts=consts,
                constf=constf,
                gidx=np.ascontiguousarray(pp["gidx"][c]),
                drel=np.ascontiguousarray(pp["drel"][c]),
                drel2=np.ascontiguousarray(pp["drel2"][c]),
                dstF=np.ascontiguousarray(pp["dstF"][c]),
            ))
        _plan_cache[key][3] = in_maps
    else:
        in_maps = in_maps_cached

    from concourse.bass_utils import run_bass_kernel_spmd
    res = run_bass_kernel_spmd(nc, in_maps, core_ids=list(range(NC)))
    _last_exec_ns[0] = getattr(res, "exec_time_ns", None)
    if debug:
        global _last_debug
        _last_debug = dict(
            tbl1=res.results[0].get("dtbl"),
            xr1=[res.results[c].get("dxr") for c in range(NC)],
            h1=[res.results[c].get("delu") for c in range(NC)],
            xl=[res.results[c].get("dxl") for c in range(NC)],
            raw0={k: v for k, v in res.results[0].items()},
        )

    h = np.empty((N, CO), np.float32)
    ls = np.empty((N, CO), np.float32)
    r_core = pp["core_of"]
    r_loc = pp["local_of"]
    for c in range(NC):
        m = r_core == c
        h[m] = res.results[c]["h2o"][r_loc[m]]
        ls[m] = res.results[c]["lso"][r_loc[m]]
    return h, ls
